# revision 3
# baseline (speedup 1.0000x reference)
"""Trainium2 Bass kernel for nn_CrossEntropyMoreToMore.

Math: out[i, n] = sum_c softplus(pre_cls[n, c]) - pre_cls[n, gt_kind_ind[i]]
with M = N = 8192, C = 80.

Key structure: there are only C=80 distinct output rows. Define
    D[c, n] = base[n] - pre_cls[n, c],  base[n] = sum_c softplus(pre_cls[n, c])
then out[i, :] = D[g[i], :].

Per-core plan (core k owns output rows [k*1024, (k+1)*1024)):
  1. Load pre_cls (replicated), compute softplus-sum base and the table
     D^T in natural [n-partition, c-free] layout, then PE-transpose the
     64 [128, 80] tiles into D [80 partitions, 8192 free] in SBUF.
  2. Build a one-hot selection matrix onehotT[c, m] = (g[m] == c) for the
     core's 1024 rows (iota + is_equal + PE transpose).
  3. For each [128 m, 512 n] output tile: fp32 matmul
     out_tile = onehotT[:, mtile].T @ D[:, nchunk]  (exact: one-hot weights)
     then copy PSUM->SBUF and DMA the tile to DRAM.

This makes HBM traffic per core = 32 MB of output writes + 2.6 MB of input
reads (the memory roofline), with the row-gather done by the PE.
"""

import numpy as np

M, N, C = 8192, 8192, 80
N_CORES = 8
M_SHARD = M // N_CORES  # 1024 output rows per core
P = 128  # partitions
NT = N // P  # 64 column tiles of pre_cls
MT = M_SHARD // P  # 8 m-tiles per core
NCHUNK = 512  # matmul moving-dim size (one PSUM bank of fp32)
NJ = N // NCHUNK  # 16 n-chunks

_compiled_nc = None


def _build_kernel():
    import concourse.bacc as bacc
    import concourse.mybir as mybir
    import concourse.tile as tile
    from concourse.masks import make_identity

    nc = bacc.Bacc(
        "TRN2",
        target_bir_lowering=False,
        debug=False,
        num_devices=N_CORES,
    )
    fp32 = mybir.dt.float32
    i32 = mybir.dt.int32

    g_dram = nc.dram_tensor("g", [M_SHARD], fp32, kind="ExternalInput")
    pre_dram = nc.dram_tensor("pre", [N, C], fp32, kind="ExternalInput")
    out_dram = nc.dram_tensor("out", [M_SHARD, N], fp32, kind="ExternalOutput")

    with tile.TileContext(nc) as tc:
        with (
            tc.tile_pool(name="setup", bufs=1) as setup,
            tc.tile_pool(name="stage", bufs=6) as stage,
            tc.tile_pool(name="psum_tr", bufs=2, space="PSUM") as psum_tr,
            tc.tile_pool(name="psum_mm", bufs=4, space="PSUM") as psum_mm,
        ):
            ident = setup.tile([P, P], fp32)
            make_identity(nc, ident[:])

            # ---- base[n] and the D^T table in natural layout ----
            pre_t = setup.tile([P, NT, C], fp32)
            nc.sync.dma_start(
                pre_t[:], pre_dram.ap().rearrange("(t p) c -> p t c", p=P)
            )
            # softplus(x) = relu(x) + ln(1 + exp(-|x|)); Softplus has no ACT
            # table in this build, so compose it from Abs/Exp/Ln/Relu (all
            # within the natural_log_exp_and_others table).
            t0 = setup.tile([P, NT, C], fp32)
            nc.scalar.activation(t0[:], pre_t[:], mybir.ActivationFunctionType.Abs)
            nc.scalar.activation(
                t0[:], t0[:], mybir.ActivationFunctionType.Exp, scale=-1.0
            )
            nc.scalar.activation(
                t0[:], t0[:], mybir.ActivationFunctionType.Ln, bias=1.0
            )
            rx = setup.tile([P, NT, C], fp32)
            nc.vector.tensor_scalar_max(rx[:], pre_t[:], 0.0)
            sp = setup.tile([P, NT, C], fp32)
            nc.vector.tensor_add(sp[:], t0[:], rx[:])
            base = setup.tile([P, NT, 1], fp32)
            nc.vector.reduce_sum(base[:], sp[:], axis=mybir.AxisListType.X)
            # dtt[p, t, c] = base[p, t] - pre[p, t, c]
            dtt = setup.tile([P, NT, C], fp32)
            nc.vector.tensor_tensor(
                out=dtt[:],
                in0=base[:].to_broadcast([P, NT, C]),
                in1=pre_t[:],
                op=mybir.AluOpType.subtract,
            )

            # ---- transpose D^T tiles into D [80, 8192] ----
            d_table = setup.tile([C, N], fp32)
            for t in range(NT):
                ps = psum_tr.tile([C, P], fp32)
                nc.tensor.transpose(ps[:], dtt[:, t, :], ident[:])
                nc.scalar.copy(d_table[:, t * P : (t + 1) * P], ps[:])

            # ---- one-hot selection matrix [80, 1024] ----
            g_col = setup.tile([P, MT], fp32)
            nc.sync.dma_start(g_col[:], g_dram.ap().rearrange("(t p) -> p t", p=P))
            iota_row = setup.tile([P, C], fp32)
            nc.gpsimd.iota(
                iota_row[:],
                pattern=[[1, C]],
                channel_multiplier=0,
                allow_small_or_imprecise_dtypes=True,
            )
            onehot = setup.tile([C, M_SHARD], fp32)
            for i in range(MT):
                rowhot = stage.tile([P, C], fp32, tag="rowhot")
                nc.vector.tensor_scalar(
                    out=rowhot[:],
                    in0=iota_row[:],
                    scalar1=g_col[:, i : i + 1],
                    scalar2=None,
                    op0=mybir.AluOpType.is_equal,
                )
                ps = psum_tr.tile([C, P], fp32)
                nc.tensor.transpose(ps[:], rowhot[:], ident[:])
                nc.scalar.copy(onehot[:, i * P : (i + 1) * P], ps[:])

            # ---- main loop: out tile = onehot_mtile.T @ D_nchunk ----
            for j in range(NJ):
                for i in range(MT):
                    ps = psum_mm.tile([P, NCHUNK], fp32)
                    nc.tensor.matmul(
                        ps[:],
                        lhsT=onehot[:, i * P : (i + 1) * P],
                        rhs=d_table[:, j * NCHUNK : (j + 1) * NCHUNK],
                        start=True,
                        stop=True,
                    )
                    st = stage.tile([P, NCHUNK], fp32, tag="st")
                    if (i + j) % 2 == 0:
                        nc.vector.tensor_copy(st[:], ps[:])
                    else:
                        nc.scalar.copy(st[:], ps[:])
                    nc.sync.dma_start(
                        out_dram.ap()[
                            i * P : (i + 1) * P, j * NCHUNK : (j + 1) * NCHUNK
                        ],
                        st[:],
                    )

    nc.compile()
    return nc


def _get_nc():
    global _compiled_nc
    if _compiled_nc is None:
        _compiled_nc = _build_kernel()
    return _compiled_nc


def _in_maps(gt_kind_ind, pre_cls):
    g = np.ascontiguousarray(np.asarray(gt_kind_ind).astype(np.float32))
    pre = np.ascontiguousarray(np.asarray(pre_cls, dtype=np.float32))
    assert g.shape == (M,) and pre.shape == (N, C)
    return [
        {"g": g[k * M_SHARD : (k + 1) * M_SHARD], "pre": pre}
        for k in range(N_CORES)
    ]


def kernel(gt_kind_ind, pre_cls, _trace=False):
    from concourse.bass_utils import run_bass_kernel_spmd

    nc = _get_nc()
    res = run_bass_kernel_spmd(
        nc, _in_maps(gt_kind_ind, pre_cls), list(range(N_CORES)), trace=_trace
    )
    out = np.concatenate(
        [res.results[k]["out"] for k in range(N_CORES)], axis=0
    )
    if _trace:
        return out, res
    return out


# revision 6
# speedup vs baseline: 1.0011x; 1.0011x over previous
"""Trainium2 Bass kernel for nn_CrossEntropyMoreToMore.

Math: out[i, n] = sum_c softplus(pre_cls[n, c]) - pre_cls[n, gt_kind_ind[i]]
with M = N = 8192, C = 80.

Key structure: there are only C=80 distinct output rows. Define
    D[c, n] = base[n] - pre_cls[n, c],  base[n] = sum_c softplus(pre_cls[n, c])
then out[i, :] = D[g[i], :].

Per-core plan (core k owns output rows [k*1024, (k+1)*1024)):
  1. Load pre_cls (replicated), compute softplus-sum base and the table
     D^T in natural [n-partition, c-free] layout, then PE-transpose the
     64 [128, 80] tiles into D [80 partitions, 8192 free] in SBUF.
  2. Build a one-hot selection matrix onehotT[c, m] = (g[m] == c) for the
     core's 1024 rows (iota + is_equal + PE transpose).
  3. For each [128 m, 512 n] output tile: fp32 matmul
     out_tile = onehotT[:, mtile].T @ D[:, nchunk]  (exact: one-hot weights)
     then copy PSUM->SBUF and DMA the tile to DRAM.

This makes HBM traffic per core = 32 MB of output writes + 2.6 MB of input
reads (the memory roofline), with the row-gather done by the PE.
"""

import numpy as np

M, N, C = 8192, 8192, 80
N_CORES = 8
M_SHARD = M // N_CORES  # 1024 output rows per core
P = 128  # partitions
NT = N // P  # 64 column tiles of pre_cls
MT = M_SHARD // P  # 8 m-tiles per core
NCHUNK = 512  # matmul moving-dim size (one PSUM bank of fp32)
NJ = N // NCHUNK  # 16 n-chunks

import os

# Main-matmul operand dtype: float32r streams 4x faster through the PE than
# float32 for N>=256. Selection exactness verified empirically on HW.
MM_F32R = os.environ.get("MM_F32R", "1") != "0"
PSUM_BUFS = int(os.environ.get("PSUM_BUFS", "6"))

_compiled_nc = None


def _build_kernel():
    import concourse.bacc as bacc
    import concourse.mybir as mybir
    import concourse.tile as tile
    from concourse.masks import make_identity

    nc = bacc.Bacc(
        "TRN2",
        target_bir_lowering=False,
        debug=False,
        num_devices=N_CORES,
    )
    fp32 = mybir.dt.float32
    i32 = mybir.dt.int32

    g_dram = nc.dram_tensor("g", [M_SHARD], fp32, kind="ExternalInput")
    pre_dram = nc.dram_tensor("pre", [N, C], fp32, kind="ExternalInput")
    out_dram = nc.dram_tensor("out", [M_SHARD, N], fp32, kind="ExternalOutput")

    with tile.TileContext(nc) as tc:
        with (
            tc.tile_pool(name="setup", bufs=1) as setup,
            tc.tile_pool(name="stage", bufs=6) as stage,
            tc.tile_pool(name="psum_tr", bufs=2, space="PSUM") as psum_tr,
            tc.tile_pool(name="psum_mm", bufs=PSUM_BUFS, space="PSUM") as psum_mm,
        ):
            ident = setup.tile([P, P], fp32)
            make_identity(nc, ident[:])

            # ---- base[n] and the D^T table in natural layout ----
            pre_t = setup.tile([P, NT, C], fp32)
            nc.sync.dma_start(
                pre_t[:], pre_dram.ap().rearrange("(t p) c -> p t c", p=P)
            )
            # softplus(x) = relu(x) + ln(1 + exp(-|x|)); Softplus has no ACT
            # table in this build, so compose it from Abs/Exp/Ln/Relu (all
            # within the natural_log_exp_and_others table).
            t0 = setup.tile([P, NT, C], fp32)
            nc.scalar.activation(t0[:], pre_t[:], mybir.ActivationFunctionType.Abs)
            nc.scalar.activation(
                t0[:], t0[:], mybir.ActivationFunctionType.Exp, scale=-1.0
            )
            nc.scalar.activation(
                t0[:], t0[:], mybir.ActivationFunctionType.Ln, bias=1.0
            )
            rx = setup.tile([P, NT, C], fp32)
            nc.vector.tensor_scalar_max(rx[:], pre_t[:], 0.0)
            sp = setup.tile([P, NT, C], fp32)
            nc.vector.tensor_add(sp[:], t0[:], rx[:])
            base = setup.tile([P, NT, 1], fp32)
            nc.vector.reduce_sum(base[:], sp[:], axis=mybir.AxisListType.X)
            # dtt[p, t, c] = base[p, t] - pre[p, t, c]
            dtt = setup.tile([P, NT, C], fp32)
            nc.vector.tensor_tensor(
                out=dtt[:],
                in0=base[:].to_broadcast([P, NT, C]),
                in1=pre_t[:],
                op=mybir.AluOpType.subtract,
            )

            # ---- transpose D^T tiles into D [80, 8192] ----
            d_table = setup.tile([C, N], fp32)
            for t in range(NT):
                ps = psum_tr.tile([C, P], fp32)
                nc.tensor.transpose(ps[:], dtt[:, t, :], ident[:])
                nc.scalar.copy(d_table[:, t * P : (t + 1) * P], ps[:])

            # ---- one-hot selection matrix [80, 1024] ----
            g_col = setup.tile([P, MT], fp32)
            nc.sync.dma_start(g_col[:], g_dram.ap().rearrange("(t p) -> p t", p=P))
            iota_row = setup.tile([P, C], fp32)
            nc.gpsimd.iota(
                iota_row[:],
                pattern=[[1, C]],
                channel_multiplier=0,
                allow_small_or_imprecise_dtypes=True,
            )
            onehot = setup.tile([C, M_SHARD], fp32)
            for i in range(MT):
                rowhot = stage.tile([P, C], fp32, tag="rowhot")
                nc.vector.tensor_scalar(
                    out=rowhot[:],
                    in0=iota_row[:],
                    scalar1=g_col[:, i : i + 1],
                    scalar2=None,
                    op0=mybir.AluOpType.is_equal,
                )
                ps = psum_tr.tile([C, P], fp32)
                nc.tensor.transpose(ps[:], rowhot[:], ident[:])
                nc.scalar.copy(onehot[:, i * P : (i + 1) * P], ps[:])

            # ---- main loop: out tile = onehot_mtile.T @ D_nchunk ----
            f32r = mybir.dt.float32r
            for j in range(NJ):
                for i in range(MT):
                    ps = psum_mm.tile([P, NCHUNK], fp32)
                    lhs_ap = onehot[:, i * P : (i + 1) * P]
                    rhs_ap = d_table[:, j * NCHUNK : (j + 1) * NCHUNK]
                    if MM_F32R:
                        lhs_ap = lhs_ap.bitcast(f32r)
                        rhs_ap = rhs_ap.bitcast(f32r)
                    nc.tensor.matmul(
                        ps[:],
                        lhsT=lhs_ap,
                        rhs=rhs_ap,
                        start=True,
                        stop=True,
                    )
                    st = stage.tile([P, NCHUNK], fp32, tag="st")
                    if (i + j) % 2 == 0:
                        nc.vector.tensor_copy(st[:], ps[:])
                    else:
                        nc.scalar.copy(st[:], ps[:])
                    nc.sync.dma_start(
                        out_dram.ap()[
                            i * P : (i + 1) * P, j * NCHUNK : (j + 1) * NCHUNK
                        ],
                        st[:],
                    )

    nc.compile()
    return nc


def _get_nc():
    global _compiled_nc
    if _compiled_nc is None:
        _compiled_nc = _build_kernel()
    return _compiled_nc


def _in_maps(gt_kind_ind, pre_cls):
    g = np.ascontiguousarray(np.asarray(gt_kind_ind).astype(np.float32))
    pre = np.ascontiguousarray(np.asarray(pre_cls, dtype=np.float32))
    assert g.shape == (M,) and pre.shape == (N, C)
    return [
        {"g": g[k * M_SHARD : (k + 1) * M_SHARD], "pre": pre}
        for k in range(N_CORES)
    ]


def kernel(gt_kind_ind, pre_cls, _trace=False):
    from concourse.bass_utils import run_bass_kernel_spmd

    nc = _get_nc()
    res = run_bass_kernel_spmd(
        nc, _in_maps(gt_kind_ind, pre_cls), list(range(N_CORES)), trace=_trace
    )
    out = np.concatenate(
        [res.results[k]["out"] for k in range(N_CORES)], axis=0
    )
    if _trace:
        return out, res
    return out


# revision 11
# speedup vs baseline: 1.4554x; 1.4538x over previous
"""Trainium2 Bass kernel for nn_CrossEntropyMoreToMore.

Math: out[i, n] = sum_c softplus(pre_cls[n, c]) - pre_cls[n, gt_kind_ind[i]]
with M = N = 8192, C = 80.

Key structure: there are only C=80 distinct output rows. Define
    D[c, n] = base[n] - pre_cls[n, c],  base[n] = sum_c softplus(pre_cls[n, c])
then out[i, :] = D[g[i], :].

Per-core plan (core k owns output rows [k*1024, (k+1)*1024)):
  1. Build D as a pair of bf16 tables (hi + lo split: D = hi + lo exactly to
     ~2^-17 relative) in [class-partition, n-free] layout, pipelined in 4
     column-quarters: load pre_cls chunk -> softplus (Abs/Exp/Ln compose) ->
     reduce -> subtract -> PE-transpose -> hi/lo split.
  2. Build a bf16 one-hot selection matrix onehotT[c, m] = (g[m] == c).
  3. For each [128 m, 512 n] psum chunk: two accumulating bf16 matmuls
     (hi then lo) produce out = onehotT.T @ D exactly in fp32 PSUM;
     2048-wide PSUM->SBUF copies alternate between DVE and ACT; 2 MB DMA
     stores stream the result to HBM.

HBM traffic per core = 32 MB output writes + 2.6 MB input reads (memory
roofline ~90 us at ~358 GB/s per core).
"""

import os

import numpy as np

M, N, C = 8192, 8192, 80
N_CORES = 8
M_SHARD = M // N_CORES  # 1024 output rows per core
P = 128  # partitions
NT = N // P  # 64 column tiles of pre_cls
MT = M_SHARD // P  # 8 m-tiles per core
NCHUNK = 512  # matmul moving-dim size (one PSUM bank of fp32)
NQ = 4  # column quarters for the pipelined table build
QT = NT // NQ  # 16 transpose tiles per quarter
QW = N // NQ  # 2048 columns per quarter

W_PSUM = 2048  # psum tile width (4 banks)
SW = 4096  # staging/store width (2 MB stores)

MM_MODE = os.environ.get("MM_MODE", "bf16")

_compiled_nc = None


def _build_kernel():
    import concourse.bacc as bacc
    import concourse.mybir as mybir
    import concourse.tile as tile
    from concourse.masks import make_identity

    nc = bacc.Bacc(
        "TRN2",
        target_bir_lowering=False,
        debug=False,
        num_devices=N_CORES,
    )
    fp32 = mybir.dt.float32
    bf16 = mybir.dt.bfloat16
    AF = mybir.ActivationFunctionType
    ALU = mybir.AluOpType

    g_dram = nc.dram_tensor("g", [M_SHARD], fp32, kind="ExternalInput")
    pre_dram = nc.dram_tensor("pre", [N, C], fp32, kind="ExternalInput")
    out_dram = nc.dram_tensor("out", [M_SHARD, N], fp32, kind="ExternalOutput")

    pre_tiled = pre_dram.ap().rearrange("(t p) c -> p t c", p=P)

    with tile.TileContext(nc) as tc:
        with (
            tc.tile_pool(name="setup", bufs=1) as setup,
            tc.tile_pool(name="pipe", bufs=2) as pipe,
            tc.tile_pool(name="stage", bufs=3) as stage,
            tc.tile_pool(name="psum", bufs=2, space="PSUM") as psum,
        ):
            ident = setup.tile([P, P], fp32)
            make_identity(nc, ident[:])

            # ---- one-hot selection matrix [80, 1024] in bf16 ----
            g_col = setup.tile([P, MT], fp32)
            nc.sync.dma_start(g_col[:], g_dram.ap().rearrange("(t p) -> p t", p=P))
            iota_row = setup.tile([P, C], fp32)
            nc.gpsimd.iota(
                iota_row[:],
                pattern=[[1, C]],
                channel_multiplier=0,
                allow_small_or_imprecise_dtypes=True,
            )
            oh = setup.tile([C, M_SHARD], bf16)
            for i in range(MT):
                rowhot = pipe.tile([P, C], fp32, tag="rowhot")
                nc.vector.tensor_scalar(
                    out=rowhot[:],
                    in0=iota_row[:],
                    scalar1=g_col[:, i : i + 1],
                    scalar2=None,
                    op0=ALU.is_equal,
                )
                ps = psum.tile([C, P], fp32, tag="mm")
                nc.tensor.transpose(ps[:], rowhot[:], ident[:])
                nc.scalar.copy(oh[:, i * P : (i + 1) * P], ps[:])

            # ---- D table as bf16 hi/lo pair, built in 4 column quarters ----
            d_hi = setup.tile([C, N], bf16)
            d_lo = setup.tile([C, N], bf16)
            for Q in range(NQ):
                pre_q = pipe.tile([P, QT, C], fp32, tag="pre")
                nc.sync.dma_start(
                    pre_q[:], pre_tiled[:, Q * QT : (Q + 1) * QT, :]
                )
                # softplus(x) = relu(x) + ln(1 + exp(-|x|))
                t0 = pipe.tile([P, QT, C], fp32, tag="t0")
                nc.scalar.activation(t0[:], pre_q[:], AF.Abs)
                nc.scalar.activation(t0[:], t0[:], AF.Exp, scale=-1.0)
                nc.scalar.activation(t0[:], t0[:], AF.Ln, bias=1.0)
                rx = pipe.tile([P, QT, C], fp32, tag="rx")
                nc.vector.tensor_scalar_max(rx[:], pre_q[:], 0.0)
                nc.vector.tensor_add(rx[:], t0[:], rx[:])  # rx = softplus(pre)
                baseq = pipe.tile([P, QT, 1], fp32, tag="base")
                nc.vector.reduce_sum(baseq[:], rx[:], axis=mybir.AxisListType.X)
                # dtt[p, t, c] = base[p, t] - pre[p, t, c]  (onto t0)
                nc.vector.tensor_tensor(
                    out=t0[:],
                    in0=baseq[:].to_broadcast([P, QT, C]),
                    in1=pre_q[:],
                    op=ALU.subtract,
                )
                # transpose the 16 tiles into a f32 quarter, then split hi/lo
                dt_q = pipe.tile([C, QW], fp32, tag="dtq")
                for t in range(QT):
                    ps = psum.tile([C, P], fp32, tag="mm")
                    nc.tensor.transpose(ps[:], t0[:, t, :], ident[:])
                    nc.scalar.copy(dt_q[:, t * P : (t + 1) * P], ps[:])
                n0 = Q * QW
                nc.vector.tensor_copy(d_hi[:, n0 : n0 + QW], dt_q[:])
                nc.vector.tensor_tensor(
                    out=d_lo[:, n0 : n0 + QW],
                    in0=dt_q[:],
                    in1=d_hi[:, n0 : n0 + QW],
                    op=ALU.subtract,
                )

            # ---- main loop: out tile = onehot_mtile.T @ D_nchunk ----
            eng = 0
            for jo in range(N // SW):
                for i in range(MT):
                    st = stage.tile([P, SW], fp32, tag="st")
                    lhs = oh[:, i * P : (i + 1) * P]
                    for h in range(SW // W_PSUM):
                        pt = psum.tile([P, W_PSUM], fp32, tag="mm")
                        for q in range(W_PSUM // NCHUNK):
                            n0 = jo * SW + h * W_PSUM + q * NCHUNK
                            nc.tensor.matmul(
                                pt[:, q * NCHUNK : (q + 1) * NCHUNK],
                                lhsT=lhs,
                                rhs=d_hi[:, n0 : n0 + NCHUNK],
                                start=True,
                                stop=False,
                            )
                            nc.tensor.matmul(
                                pt[:, q * NCHUNK : (q + 1) * NCHUNK],
                                lhsT=lhs,
                                rhs=d_lo[:, n0 : n0 + NCHUNK],
                                start=False,
                                stop=True,
                            )
                        dst = st[:, h * W_PSUM : (h + 1) * W_PSUM]
                        if eng % 2 == 0:
                            nc.vector.tensor_copy(dst, pt[:])
                        else:
                            nc.scalar.copy(dst, pt[:])
                        eng += 1
                    nc.sync.dma_start(
                        out_dram.ap()[i * P : (i + 1) * P, jo * SW : (jo + 1) * SW],
                        st[:],
                    )

    nc.compile()
    return nc


def _get_nc():
    global _compiled_nc
    if _compiled_nc is None:
        _compiled_nc = _build_kernel()
    return _compiled_nc


def _in_maps(gt_kind_ind, pre_cls):
    g = np.ascontiguousarray(np.asarray(gt_kind_ind).astype(np.float32))
    pre = np.ascontiguousarray(np.asarray(pre_cls, dtype=np.float32))
    assert g.shape == (M,) and pre.shape == (N, C)
    return [
        {"g": g[k * M_SHARD : (k + 1) * M_SHARD], "pre": pre}
        for k in range(N_CORES)
    ]


def kernel(gt_kind_ind, pre_cls, _trace=False):
    from concourse.bass_utils import run_bass_kernel_spmd

    nc = _get_nc()
    res = run_bass_kernel_spmd(
        nc, _in_maps(gt_kind_ind, pre_cls), list(range(N_CORES)), trace=_trace
    )
    out = np.concatenate(
        [res.results[k]["out"] for k in range(N_CORES)], axis=0
    )
    if _trace:
        return out, res
    return out
